# revision 35
# baseline (speedup 1.0000x reference)
"""DGCNN Bass kernel for trn2 — 8-core pair-split (2 cores per sample).

Per core (half a sample: NH=2048 of N=4096 points, k=40 neighbors):
  1. kNN scores for own queries vs ALL points via K=4 fp32r matmuls.
  2. top-40: window-max pooling (8-wide) -> top-40 windows via DVE
     max/max_index rounds -> gather 40 windows (d=8) -> top-40 elements.
  3. EdgeConv rounds with gather-after-matmul factorization:
     conv([nbr-ctr, ctr]) = A[:, j] + B[:, i]; A tables over ALL points
     (rebuilt from pair-AllGathered x1/x2), B tables over own points.
  4. Global-max head with W7 split; per-pair AllGather for the global max.

BN scales folded into weights on host; LeakyReLU on the Act engine.
"""
import numpy as np
import concourse.bass as bass
import concourse.mybir as mybir
from concourse.bacc import Bacc
from concourse.tile import TileContext

F32 = mybir.dt.float32
F32R = mybir.dt.float32r
F16 = mybir.dt.float16
BF16 = mybir.dt.bfloat16
U16 = mybir.dt.uint16
I16 = mybir.dt.int16
I32 = mybir.dt.int32
AX = mybir.AxisListType
OP = mybir.AluOpType
ACTF = mybir.ActivationFunctionType

KNBR = 40
NEG = -1e30
LEAK = 0.2


def build_core(N=4096, split=2):
    """One core's program. split=2: core owns NH=N//2 points (which half is
    decided purely by the host-fed data); pair collectives exchange x1/x2/g."""
    nc = Bacc(None)
    NH = N // split
    T = NH // 128          # own tiles
    PAIRS = T // 2
    RG = [[2 * i, 2 * i + 1] for i in range(4)] if split == 2 else None

    def din(name, shape, dt=F32):
        return nc.dram_tensor(name, shape, dt, kind="ExternalInput")

    xr_d = din("xr", [4, N], F32R)
    xa_d = din("xa", [4, NH], F32R)
    a1w_d = din("a1w", [3, 64], F32R)
    b1w_d = din("b1w", [3, 64], F32R)
    b1_d = din("b1", [64, 1])
    w2t_d = din("w2t", [64, 64], BF16)
    b2_d = din("b2", [64, 1])
    a3w_d = din("a3w", [64, 64], BF16)
    b3w_d = din("b3w", [64, 64], BF16)
    b3_d = din("b3", [64, 1])
    w4t_d = din("w4t", [64, 64], BF16)
    b4_d = din("b4", [64, 1])
    a5w_d = din("a5w", [64, 64], BF16)
    b5w_d = din("b5w", [64, 64], BF16)
    b5_d = din("b5", [64, 1])
    w6t_d = din("w6t", [64, 3 * 1024], BF16)
    b6_d = din("b6", [128, 8])
    w7gt_d = din("w7gt", [128, 8 * 4 * 128])
    b7_d = din("b7", [128, 4])
    w7xt_d = din("w7xt", [64, 3 * 4 * 128], BF16)
    w8t_d = din("w8t", [128, 4 * 2 * 128], BF16)
    b8_d = din("b8", [128, 2])
    w9t_d = din("w9t", [128, 2 * 63], BF16)
    b9_d = din("b9", [63, 1])

    out_d = nc.dram_tensor("out", [63, NH], F32, kind="ExternalOutput")
    debug = bool(int(__import__("os").environ.get("DGCNN_DEBUG", "0")))
    if debug:
        idx_dbg = nc.dram_tensor("idx_dbg", [128, T * KNBR], U16,
                                 kind="ExternalOutput")
        x1_dbg = nc.dram_tensor("x1_dbg", [64, NH], F32, kind="ExternalOutput")
        x2_dbg = nc.dram_tensor("x2_dbg", [64, NH], F32, kind="ExternalOutput")
        x3_dbg = nc.dram_tensor("x3_dbg", [64, NH], F32, kind="ExternalOutput")
        g_dbg = nc.dram_tensor("g_dbg", [128, 8], F32, kind="ExternalOutput")
        xf_dbg = nc.dram_tensor("xf_dbg", [64, N], F32, kind="ExternalOutput")

    def mm_r(ps, lhsT, rhs, **kw):
        nc.tensor.matmul(ps, lhsT, rhs, **kw)

    with TileContext(nc) as tc:
        with tc.tile_pool(name="persist", bufs=1) as pp:
            x1o = pp.tile([64, NH], BF16)
            x2o = pp.tile([64, NH], BF16)
            x3o = pp.tile([64, NH], BF16)
            xf = pp.tile([64, N], BF16)        # gathered full x1 / x2 (reused)
            arep = pp.tile([128, N], F32)      # A table (dup on both halves)
            brep = pp.tile([128, NH], F32)     # B table (shift-dup)
            wraps = pp.tile([128, 320 * T], U16)
            idx128 = pp.tile([128, 320 * PAIRS], U16)
            nbr_all = pp.tile([128, T * KNBR], U16)
            iota8 = pp.tile([128, 8], U16)
            nc.gpsimd.iota(iota8[:], pattern=[[1, 8]], base=0,
                           channel_multiplier=0)
            iota8m7 = pp.tile([128, 8], I16)
            nc.vector.tensor_scalar(iota8m7[:], iota8[:], 7, None,
                                    op0=OP.subtract)
            iotaJ = pp.tile([128, N], U16)
            nc.gpsimd.iota(iotaJ[:], pattern=[[1, N]], base=0,
                           channel_multiplier=0)
            iotaR1 = pp.tile([128, KNBR], U16)
            nc.gpsimd.iota(iotaR1[:], pattern=[[1, KNBR]], base=1,
                           channel_multiplier=0)
            zc = pp.tile([128, 1], F32)
            nc.gpsimd.memset(zc[:], 0.0)
            iotaCAP = pp.tile([128, 336], F32)
            nc.gpsimd.iota(iotaCAP[:], pattern=[[1, 336]], base=0,
                           channel_multiplier=0,
                           allow_small_or_imprecise_dtypes=True)

            # =====================================================
            # Stage A: kNN + top-40 per tile
            # =====================================================
            xr = pp.tile([4, N], F32R, name="xr")
            nc.sync.dma_start(out=xr[:], in_=xr_d[:])
            xa = pp.tile([4, NH], F32R, name="xa")
            nc.sync.dma_start(out=xa[:], in_=xa_d[:])
            with tc.tile_pool(name="sel_sb", bufs=2) as sp, \
                 tc.tile_pool(name="sel_ps", bufs=2, space="PSUM") as sps:
                for t in range(T):
                    s_sb = sp.tile([128, N], F32, tag="s_sb")
                    pooled = sp.tile([128, N // 8], F32, tag="pooled")
                    for h in range(2):
                        ps = sps.tile([128, N // 2], F32, tag="score")
                        for j in range(N // 2 // 512):
                            col = h * (N // 2) + j * 512
                            mm_r(ps[:, j * 512:(j + 1) * 512],
                                 xa[:, t * 128:(t + 1) * 128],
                                 xr[:, col:col + 512],
                                 start=True, stop=True)
                        nc.scalar.copy(out=s_sb[:, h * (N // 2):(h + 1) * (N // 2)],
                                       in_=ps[:])
                        # 8-wide window max (DVE; Pool has no ALU path)
                        W = N // 16          # windows per half
                        nc.vector.tensor_reduce(
                            out=pooled[:, h * W:(h + 1) * W],
                            in_=ps.rearrange("p (w k) -> p w k", k=8),
                            axis=AX.X, op=OP.max)
                    # tau = 40th-largest window max; the match_replace NEG
                    # marks left in `pooled` identify the top-40 windows
                    NW = N // 8
                    CAP = 336        # 42 candidate windows * 8 slots
                    m8 = sp.tile([128, 8], F32, tag="m8")
                    for r in range(5):
                        nc.vector.max(out=m8[:], in_=pooled[:])
                        nc.vector.match_replace(out=pooled[:], in_to_replace=m8[:],
                                                in_values=pooled[:], imm_value=NEG)
                    tau = m8[:, 7:8]
                    wmask = sp.tile([128, NW], BF16, tag="wmask")
                    nc.vector.tensor_scalar(wmask[:], pooled[:], -1e29, None,
                                            op0=OP.is_le)
                    wrank = sp.tile([128, NW], F32, tag="wrank")
                    nc.vector.tensor_tensor_scan(
                        wrank[:], wmask[:], zc.broadcast_to([128, NW]), 0.0,
                        op0=OP.add, op1=OP.add)
                    wslot8 = sp.tile([128, NW], I16, tag="wslot8")
                    nc.vector.scalar_tensor_tensor(
                        out=wslot8[:], in0=wrank[:], scalar=8.0, in1=wmask[:],
                        op0=OP.mult, op1=OP.mult)
                    # element slots: eslot[w*8+j] = wslot8[w] + j - 7
                    # (non-candidate windows give j-7 <= 0 -> dropped)
                    eslot = sp.tile([128, N], I16, tag="eslot")
                    nc.vector.tensor_tensor(
                        out=eslot.rearrange("p (w j) -> p w j", j=8),
                        in0=wslot8.unsqueeze(2).broadcast_to([128, NW, 8]),
                        in1=iota8m7.unsqueeze(1).broadcast_to([128, NW, 8]),
                        op=OP.add)
                    # shifted scores: boundary at 0 keeps f16 ordering exact
                    sb1 = sp.tile([128, N], F16, tag="sb1")
                    negtau1 = sp.tile([128, 1], F32, tag="negtau1")
                    nc.vector.tensor_scalar(negtau1[:], tau, -1.0, None,
                                            op0=OP.mult)
                    nc.scalar.activation(sb1[:], s_sb[:], ACTF.Identity,
                                         bias=negtau1[:])
                    candh = sp.tile([128, CAP], F16, tag="candh")
                    nc.gpsimd.local_scatter(candh[:], sb1[:], eslot[:],
                                            channels=128, num_elems=CAP,
                                            num_idxs=N)
                    candidx = sp.tile([128, CAP], U16, tag="candidx")
                    nc.gpsimd.local_scatter(candidx[:], iotaJ[:], eslot[:],
                                            channels=128, num_elems=CAP,
                                            num_idxs=N)
                    # mask empty slots (slot > 8*n_cand_windows) to NEG
                    count8 = sp.tile([128, 1], F32, tag="count8")
                    nc.vector.tensor_scalar(count8[:], wrank[:, NW - 1:NW], 8.0,
                                            None, op0=OP.mult)
                    emptym = sp.tile([128, CAP], F32, tag="emptym")
                    nc.vector.tensor_scalar(emptym[:], iotaCAP[:], count8[:],
                                            None, op0=OP.is_gt)
                    candf = sp.tile([128, CAP], F32, tag="candf")
                    nc.vector.scalar_tensor_tensor(
                        out=candf[:], in0=emptym[:], scalar=NEG, in1=candh[:],
                        op0=OP.mult, op1=OP.add)
                    pos40 = sp.tile([128, KNBR], U16, tag="pos40")
                    cv = candf[:, 1:CAP]
                    for r in range(5):
                        nc.vector.max(out=m8[:], in_=cv)
                        nc.vector.max_index(out=pos40[:, r * 8:(r + 1) * 8],
                                            in_max=m8[:], in_values=cv)
                        nc.vector.match_replace(out=cv, in_to_replace=m8[:],
                                                in_values=cv, imm_value=NEG)
                    # map candidate positions back to global element ids
                    posi = sp.tile([128, KNBR], I16, tag="posi")
                    nc.vector.tensor_scalar(posi[:], pos40[:], 1, None, op0=OP.add)
                    winv = sp.tile([128, CAP], U16, tag="winv")
                    nc.gpsimd.local_scatter(winv[:], iotaR1[:], posi[:],
                                            channels=128, num_elems=CAP,
                                            num_idxs=KNBR)
                    winm1 = sp.tile([128, CAP], I16, tag="winm1")
                    nc.vector.tensor_scalar(winm1[:], winv[:], 1, None,
                                            op0=OP.subtract)
                    nc.gpsimd.local_scatter(
                        nbr_all[:, t * KNBR:(t + 1) * KNBR], candidx[:], winm1[:],
                        channels=128, num_elems=KNBR, num_idxs=CAP)
                    # wrapped list: wrap[ilo, kk*8+ihi] = nbr[ihi*16+ilo, kk]
                    pbase = 64 * (t % 2)
                    for ihi in range(8):
                        dst = wraps[pbase:pbase + 16, t * 320:(t + 1) * 320] \
                            .rearrange("p (k e) -> p k e", e=8)[:, :, ihi:ihi + 1]
                        nc.sync.dma_start(
                            out=dst,
                            in_=nbr_all[ihi * 16:(ihi + 1) * 16,
                                        t * KNBR:(t + 1) * KNBR].unsqueeze(2))
                    # replicate to the second quadrant (+32 partitions)
                    nc.sync.dma_start(
                        out=wraps[pbase + 32: pbase + 48, t * 320:(t + 1) * 320],
                        in_=wraps[pbase: pbase + 16, t * 320:(t + 1) * 320])
                # shuffle wrapped lists into per-pair gather index lists (once)
                rep16 = [i % 16 for i in range(32)]
                for p in range(PAIRS):
                    tA = 2 * p
                    nc.vector.stream_shuffle(
                        idx128[0:64, p * 320:(p + 1) * 320],
                        wraps[0:64, tA * 320:(tA + 1) * 320], mask=rep16)
                    nc.vector.stream_shuffle(
                        idx128[64:128, p * 320:(p + 1) * 320],
                        wraps[64:128, (tA + 1) * 320:(tA + 2) * 320], mask=rep16)

            # =====================================================
            # EdgeConv machinery
            # =====================================================
            def build_a_table(aw_t, src_full, kdim, fp32=False, order=None):
                """A table over ALL N points -> arep (f32, duplicated halves).
                order: chunk emission order (readiness after split AllGathers)."""
                with tc.tile_pool(name="ta_ps", bufs=2, space="PSUM") as tps:
                    for j in (order or range(N // 512)):
                        psa = tps.tile([64, 512], F32, tag="psa")
                        if fp32:
                            mm_r(psa[:], aw_t[:], src_full[:, j * 512:(j + 1) * 512],
                                 start=True, stop=True)
                        else:
                            nc.tensor.matmul(psa[:], aw_t[:],
                                             src_full[:, j * 512:(j + 1) * 512],
                                             start=True, stop=True)
                        nc.scalar.copy(out=arep[0:64, j * 512:(j + 1) * 512],
                                       in_=psa[:])
                        nc.sync.dma_start(out=arep[64:128, j * 512:(j + 1) * 512],
                                          in_=arep[0:64, j * 512:(j + 1) * 512])

            def build_b_table(bw_t, bias_t, src_own, fp32=False):
                """B table (+bias) over OWN points -> brep (f32, shift-dup)."""
                with tc.tile_pool(name="tb_ps", bufs=2, space="PSUM") as tps:
                    for j in range(NH // 512):
                        psb = tps.tile([64, 512], F32, tag="psb")
                        if fp32:
                            mm_r(psb[:], bw_t[:], src_own[:, j * 512:(j + 1) * 512],
                                 start=True, stop=True)
                        else:
                            nc.tensor.matmul(psb[:], bw_t[:],
                                             src_own[:, j * 512:(j + 1) * 512],
                                             start=True, stop=True)
                        nc.scalar.activation(brep[0:64, j * 512:(j + 1) * 512],
                                             psb[:], ACTF.Identity, bias=bias_t[:])
                    nc.sync.dma_start(out=brep[64:128, 0:NH - 128],
                                      in_=brep[0:64, 128:NH])

            def edge_round(wt_t, bias_t, xout, last=False, half_hook=None):
                with tc.tile_pool(name="er_sb", bufs=2) as esp, \
                     tc.tile_pool(name="er_ps", bufs=2, space="PSUM") as eps:
                    for p in range(PAIRS):
                        if half_hook is not None and p == PAIRS // 2:
                            half_hook(0)
                        tA = 2 * p
                        ga = esp.tile([128, KNBR * 128], F32, tag="ga")
                        nc.gpsimd.ap_gather(
                            ga[:], arep[:],
                            idx128[:, p * 320:(p + 1) * 320].bitcast(I16),
                            channels=128, num_elems=N, d=1, num_idxs=KNBR * 128)
                        if last:
                            mx = esp.tile([128, 128], F32, tag="mx")
                            nc.vector.tensor_reduce(
                                out=mx[:], in_=ga.rearrange("p (k i) -> p i k", k=KNBR),
                                axis=AX.X, op=OP.max)
                            zz = esp.tile([128, 128], F32, tag="zz")
                            nc.vector.tensor_tensor(
                                out=zz[:], in0=mx[:],
                                in1=brep[:, tA * 128: tA * 128 + 128], op=OP.add)
                            xo = esp.tile([128, 128], BF16, tag="xo")
                            nc.vector.scalar_tensor_tensor(
                                out=xo[:], in0=zz[:], scalar=LEAK, in1=zz[:],
                                op0=OP.mult, op1=OP.max)
                        else:
                            bview = brep[:, tA * 128: tA * 128 + 128] \
                                .unsqueeze(1).broadcast_to([128, KNBR, 128])
                            e = esp.tile([128, KNBR * 128], BF16, tag="e")
                            nc.vector.tensor_tensor(
                                out=e.rearrange("p (k i) -> p k i", k=KNBR),
                                in0=ga.rearrange("p (k i) -> p k i", k=KNBR),
                                in1=bview, op=OP.add)
                            nc.vector.scalar_tensor_tensor(
                                out=e[:], in0=e[:], scalar=LEAK, in1=e[:],
                                op0=OP.mult, op1=OP.max)
                            # conv2 in 2048-col PSUM chunks (4 banks each);
                            # per-chunk k=16 max-reduce on DVE
                            NCH = 3
                            widths = [2048, 2048, 1024]
                            pmax = esp.tile([128, NCH * 128], F32, tag="pmax")
                            col = 0
                            for c, wdt in enumerate(widths):
                                cps_t = eps.tile([128, 2048], F32, tag="cps")
                                cps = cps_t[:, 0:wdt]
                                for j in range(wdt // 512):
                                    for hh in range(2):
                                        pb = 64 * hh
                                        nc.tensor.matmul(
                                            cps[pb:pb + 64, j * 512:(j + 1) * 512],
                                            wt_t[pb:pb + 64, :],
                                            e[pb:pb + 64, col + j * 512:
                                              col + (j + 1) * 512],
                                            start=True, stop=True)
                                nc.vector.tensor_reduce(
                                    out=pmax[:, c * 128:(c + 1) * 128],
                                    in_=cps.rearrange("p (k i) -> p i k", k=wdt // 128),
                                    axis=AX.X, op=OP.max)
                                col += wdt
                            mx = esp.tile([128, 128], F32, tag="mx")
                            nc.vector.tensor_reduce(
                                out=mx[:], in_=pmax.rearrange("p (k i) -> p i k", k=NCH),
                                axis=AX.X, op=OP.max)
                            xz = esp.tile([128, 128], F32, tag="xz")
                            nc.scalar.activation(xz[:], mx[:], ACTF.Identity,
                                                 bias=bias_t[:])
                            xo = esp.tile([128, 128], BF16, tag="xo")
                            nc.vector.scalar_tensor_tensor(
                                out=xo[:], in0=xz[:], scalar=LEAK, in1=xz[:],
                                op0=OP.mult, op1=OP.max)
                        nc.sync.dma_start(out=xout[:, tA * 128:(tA + 1) * 128],
                                          in_=xo[0:64, :])
                        nc.sync.dma_start(out=xout[:, (tA + 1) * 128:(tA + 2) * 128],
                                          in_=xo[64:128, :])
                    if half_hook is not None:
                        half_hook(1)

            def load_bias128(bias_d_, pool, tag):
                bt = pool.tile([128, 1], F32, tag=tag)
                nc.sync.dma_start(out=bt[0:64, :], in_=bias_d_[:])
                nc.sync.dma_start(out=bt[64:128, :], in_=bias_d_[:])
                return bt

            def allgather_half(xown, hf, dram_pool, nm):
                """Pair-AllGather column-half hf of own (64, NH) bf16 into xf.
                Emitted mid-round so the collective overlaps remaining pairs."""
                HQ = NH // 2
                c0 = hf * HQ
                if split == 1:
                    nc.sync.dma_start(out=xf[:, c0:c0 + HQ],
                                      in_=xown[:, c0:c0 + HQ])
                    return
                gin = dram_pool.tile([64, HQ], BF16, name=f"gin_{nm}{hf}")
                gout = dram_pool.tile([128, HQ], BF16, name=f"gout_{nm}{hf}")
                nc.sync.dma_start(out=gin[:], in_=xown[:, c0:c0 + HQ])
                nc.gpsimd.collective_compute(
                    "AllGather", OP.bypass, replica_groups=RG,
                    ins=[gin[:]], outs=[gout[:]])
                nc.sync.dma_start(out=xf[:, c0:c0 + HQ], in_=gout[0:64, :])
                nc.sync.dma_start(out=xf[:, NH + c0:NH + c0 + HQ],
                                  in_=gout[64:128, :])

            with tc.tile_pool(name="wts", bufs=1) as wp, \
                 tc.tile_pool(name="dram", bufs=1, space="DRAM") as dram:
                # ---- EdgeConv 1 (tables from coordinates, fp32) ----
                a1w = wp.tile([3, 64], F32R)
                nc.sync.dma_start(out=a1w[:], in_=a1w_d[:])
                b1w = wp.tile([3, 64], F32R)
                nc.sync.dma_start(out=b1w[:], in_=b1w_d[:])
                b1t = wp.tile([64, 1], F32)
                nc.sync.dma_start(out=b1t[:], in_=b1_d[:])
                w2t = wp.tile([128, 64], BF16)
                nc.sync.dma_start(out=w2t[0:64, :], in_=w2t_d[:])
                nc.sync.dma_start(out=w2t[64:128, :], in_=w2t_d[:])
                b2r = load_bias128(b2_d, wp, "b2r")
                build_a_table(a1w, xr[0:3, :], 3, fp32=True)
                build_b_table(b1w, b1t, xa[0:3, :], fp32=True)
                # weight loads for later rounds (overlap with EC1 compute)
                a3w = wp.tile([64, 64], BF16)
                nc.sync.dma_start(out=a3w[:], in_=a3w_d[:])
                b3w = wp.tile([64, 64], BF16)
                nc.sync.dma_start(out=b3w[:], in_=b3w_d[:])
                b3t = wp.tile([64, 1], F32)
                nc.sync.dma_start(out=b3t[:], in_=b3_d[:])
                w4t = wp.tile([128, 64], BF16)
                nc.sync.dma_start(out=w4t[0:64, :], in_=w4t_d[:])
                nc.sync.dma_start(out=w4t[64:128, :], in_=w4t_d[:])
                b4r = load_bias128(b4_d, wp, "b4r")
                a5w = wp.tile([64, 64], BF16)
                nc.sync.dma_start(out=a5w[:], in_=a5w_d[:])
                b5w = wp.tile([64, 64], BF16)
                nc.sync.dma_start(out=b5w[:], in_=b5w_d[:])
                b5t = wp.tile([64, 1], F32)
                nc.sync.dma_start(out=b5t[:], in_=b5_d[:])

                READY = [0, 1, 4, 5, 2, 3, 6, 7]   # xf chunks by AG-half order
                edge_round(w2t, b2r, x1o,
                           half_hook=lambda h: allgather_half(x1o, h, dram, "x1"))
                build_b_table(b3w, b3t, x1o)
                build_a_table(a3w, xf, 64, order=READY)
                edge_round(w4t, b4r, x2o,
                           half_hook=lambda h: allgather_half(x2o, h, dram, "x2"))
                build_b_table(b5w, b5t, x2o)
                build_a_table(a5w, xf, 64, order=READY)
                edge_round(None, None, x3o, last=True)

                # =====================================================
                # Head
                # =====================================================
                with tc.tile_pool(name="hd", bufs=1) as hp, \
                     tc.tile_pool(name="hd_sb", bufs=3) as hsp, \
                     tc.tile_pool(name="hd_ps", bufs=2, space="PSUM") as hps:
                    w6t = hp.tile([64, 3 * 1024], BF16)
                    nc.sync.dma_start(out=w6t[:], in_=w6t_d[:])
                    b6t = hp.tile([128, 8], F32)
                    nc.sync.dma_start(out=b6t[:], in_=b6_d[:])
                    w7gt = hp.tile([128, 8 * 4 * 128], F32)
                    nc.sync.dma_start(out=w7gt[:], in_=w7gt_d[:])
                    b7t = hp.tile([128, 4], F32)
                    nc.sync.dma_start(out=b7t[:], in_=b7_d[:])
                    w7xt = hp.tile([64, 3 * 4 * 128], BF16)
                    nc.sync.dma_start(out=w7xt[:], in_=w7xt_d[:])
                    w8t = hp.tile([128, 4 * 2 * 128], BF16)
                    nc.sync.dma_start(out=w8t[:], in_=w8t_d[:])
                    b8t = hp.tile([128, 2], F32)
                    nc.sync.dma_start(out=b8t[:], in_=b8_d[:])
                    w9t = hp.tile([128, 2 * 63], BF16)
                    nc.sync.dma_start(out=w9t[:], in_=w9t_d[:])
                    b9t = hp.tile([63, 1], F32)
                    nc.sync.dma_start(out=b9t[:], in_=b9_d[:])

                    if debug:
                        nc.sync.dma_start(out=idx_dbg[:], in_=nbr_all[:])
                        for src, dst in [(x1o, x1_dbg), (x2o, x2_dbg),
                                         (x3o, x3_dbg)]:
                            db = hp.tile([64, NH], F32, name=f"db_{dst.name}")
                            nc.vector.tensor_scalar(db[:], src[:], 0.0, None,
                                                    op0=OP.add)
                            nc.sync.dma_start(out=dst[:], in_=db[:])
                        dbf = hp.tile([64, N], F32, name="dbf")
                        nc.vector.tensor_scalar(dbf[:], xf[:], 0.0, None,
                                                op0=OP.add)
                        nc.sync.dma_start(out=xf_dbg[:], in_=dbf[:])
                    xs_ = [x1o, x2o, x3o]
                    NC6 = NH // 512
                    gtmp = hp.tile([128, 8 * NC6], F32)
                    for o in range(8):
                        for n in range(NC6):
                            ps6 = hps.tile([128, 512], F32, tag="ps6")
                            for kp in range(3):
                                nc.tensor.matmul(
                                    ps6[:],
                                    w6t[:, kp * 1024 + o * 128: kp * 1024 + (o + 1) * 128],
                                    xs_[kp][:, n * 512:(n + 1) * 512],
                                    start=(kp == 0), stop=(kp == 2))
                            nc.vector.tensor_reduce(
                                out=gtmp[:, o * NC6 + n: o * NC6 + n + 1],
                                in_=ps6[:], axis=AX.X, op=OP.max)
                    gpart = hp.tile([128, 8], F32)
                    nc.vector.tensor_reduce(
                        out=gpart[:], in_=gtmp.rearrange("p (o n) -> p o n", o=8),
                        axis=AX.X, op=OP.max)
                    g = hp.tile([128, 8], F32)
                    if split == 2:
                        ggin = dram.tile([128, 8], F32, name="ggin")
                        ggout = dram.tile([256, 8], F32, name="ggout")
                        nc.sync.dma_start(out=ggin[:], in_=gpart[:])
                        nc.gpsimd.collective_compute(
                            "AllGather", OP.bypass, replica_groups=RG,
                            ins=[ggin[:]], outs=[ggout[:]])
                        gA = hp.tile([128, 8], F32)
                        nc.sync.dma_start(out=gA[:], in_=ggout[0:128, :])
                        gB = hp.tile([128, 8], F32)
                        nc.sync.dma_start(out=gB[:], in_=ggout[128:256, :])
                        nc.vector.tensor_tensor(out=g[:], in0=gA[:], in1=gB[:],
                                                op=OP.max)
                    else:
                        nc.vector.tensor_tensor(out=g[:], in0=gpart[:],
                                                in1=gpart[:], op=OP.max)
                    nc.vector.tensor_tensor(out=g[:], in0=g[:], in1=b6t[:], op=OP.add)
                    g2 = hp.tile([128, 8], F32)
                    nc.vector.scalar_tensor_tensor(
                        out=g2[:], in0=g[:], scalar=LEAK, in1=g[:],
                        op0=OP.mult, op1=OP.max)
                    if debug:
                        nc.sync.dma_start(out=g_dbg[:], in_=g2[:])

                    ps7v = hps.tile([128, 4], F32, tag="ps7v", bufs=1)
                    for m in range(4):
                        for o in range(8):
                            nc.tensor.matmul(
                                ps7v[:, m:m + 1],
                                w7gt[:, (o * 4 + m) * 128:(o * 4 + m + 1) * 128],
                                g2[:, o:o + 1], start=(o == 0), stop=(o == 7))
                    v7 = hp.tile([128, 4], F32)
                    nc.vector.tensor_tensor(out=v7[:], in0=ps7v[:], in1=b7t[:],
                                            op=OP.add)

                    for n in range(NC6):
                        y7 = hsp.tile([128, 4 * 512], BF16, tag="y7")
                        for m in range(4):
                            ps7 = hps.tile([128, 512], F32, tag="ps7")
                            for kp in range(3):
                                nc.tensor.matmul(
                                    ps7[:],
                                    w7xt[:, (kp * 4 + m) * 128:(kp * 4 + m + 1) * 128],
                                    xs_[kp][:, n * 512:(n + 1) * 512],
                                    start=(kp == 0), stop=(kp == 2))
                            t7 = hsp.tile([128, 512], F32, tag="t7")
                            nc.scalar.activation(t7[:], ps7[:], ACTF.Identity,
                                                 bias=v7[:, m:m + 1])
                            nc.vector.scalar_tensor_tensor(
                                out=y7[:, m * 512:(m + 1) * 512], in0=t7[:],
                                scalar=LEAK, in1=t7[:], op0=OP.mult, op1=OP.max)
                        y8 = hsp.tile([128, 2 * 512], BF16, tag="y8")
                        for m in range(2):
                            ps8 = hps.tile([128, 512], F32, tag="ps8")
                            for k in range(4):
                                nc.tensor.matmul(
                                    ps8[:], w8t[:, (k * 2 + m) * 128:(k * 2 + m + 1) * 128],
                                    y7[:, k * 512:(k + 1) * 512],
                                    start=(k == 0), stop=(k == 3))
                            t8 = hsp.tile([128, 512], F32, tag="t7")
                            nc.scalar.activation(t8[:], ps8[:], ACTF.Identity,
                                                 bias=b8t[:, m:m + 1])
                            nc.vector.scalar_tensor_tensor(
                                out=y8[:, m * 512:(m + 1) * 512], in0=t8[:],
                                scalar=LEAK, in1=t8[:], op0=OP.mult, op1=OP.max)
                        ps9 = hps.tile([63, 512], F32, tag="ps9", bufs=1)
                        for k in range(2):
                            nc.tensor.matmul(ps9[:], w9t[:, k * 63:(k + 1) * 63],
                                             y8[:, k * 512:(k + 1) * 512],
                                             start=(k == 0), stop=(k == 1))
                        o9 = hsp.tile([63, 512], F32, tag="o9")
                        nc.scalar.activation(o9[:], ps9[:], ACTF.Identity,
                                             bias=b9t[:])
                        nc.sync.dma_start(out=out_d[:, n * 512:(n + 1) * 512],
                                          in_=o9[:])

    nc.finalize()
    return nc


# =====================================================================
# Host-side folding
# =====================================================================
def fold_weights(inp):
    """inp: the reference setup_inputs() dict. Returns dict of shared
    (sample-independent) device arrays."""
    def f64(a):
        return np.asarray(a, np.float64)

    out = {}
    W1, s1, b1 = f64(inp["W1"]), f64(inp["s1"]), f64(inp["b1"])
    W1a, W1b = W1[:, :3], W1[:, 3:]
    out["a1w"] = (s1[:, None] * W1a).T.astype(np.float32).copy()
    out["b1w"] = (s1[:, None] * (W1b - W1a)).T.astype(np.float32).copy()
    out["b1"] = b1[:, None].astype(np.float32)
    W2, s2, b2 = f64(inp["W2"]), f64(inp["s2"]), f64(inp["b2"])
    assert (s2 > 0).all()
    out["w2t"] = (s2[:, None] * W2).T.astype(np.float32).copy()
    out["b2"] = b2[:, None].astype(np.float32)
    W3, s3, b3 = f64(inp["W3"]), f64(inp["s3"]), f64(inp["b3"])
    W3a, W3b = W3[:, :64], W3[:, 64:]
    out["a3w"] = (s3[:, None] * W3a).T.astype(np.float32).copy()
    out["b3w"] = (s3[:, None] * (W3b - W3a)).T.astype(np.float32).copy()
    out["b3"] = b3[:, None].astype(np.float32)
    W4, s4, b4 = f64(inp["W4"]), f64(inp["s4"]), f64(inp["b4"])
    assert (s4 > 0).all()
    out["w4t"] = (s4[:, None] * W4).T.astype(np.float32).copy()
    out["b4"] = b4[:, None].astype(np.float32)
    W5, s5, b5 = f64(inp["W5"]), f64(inp["s5"]), f64(inp["b5"])
    W5a, W5b = W5[:, :64], W5[:, 64:]
    out["a5w"] = (s5[:, None] * W5a).T.astype(np.float32).copy()
    out["b5w"] = (s5[:, None] * (W5b - W5a)).T.astype(np.float32).copy()
    out["b5"] = b5[:, None].astype(np.float32)
    W6, s6, b6 = f64(inp["W6"]), f64(inp["s6"]), f64(inp["b6"])
    assert (s6 > 0).all()
    W6f = s6[:, None] * W6
    out["w6t"] = W6f.T.reshape(3, 64, 1024).transpose(1, 0, 2) \
        .reshape(64, 3 * 1024).astype(np.float32).copy()
    out["b6"] = b6.reshape(8, 128).T.astype(np.float32).copy()
    W7, s7, b7 = f64(inp["W7"]), f64(inp["s7"]), f64(inp["b7"])
    W7f = s7[:, None] * W7
    W7g, W7x = W7f[:, :1024], W7f[:, 1024:]
    out["w7gt"] = W7g.T.reshape(8, 128, 4, 128).transpose(1, 0, 2, 3) \
        .reshape(128, -1).astype(np.float32).copy()
    out["b7"] = b7.reshape(4, 128).T.astype(np.float32).copy()
    out["w7xt"] = W7x.T.reshape(3, 64, 4, 128).transpose(1, 0, 2, 3) \
        .reshape(64, -1).astype(np.float32).copy()
    W8, s8, b8 = f64(inp["W8"]), f64(inp["s8"]), f64(inp["b8"])
    W8f = s8[:, None] * W8
    out["w8t"] = W8f.T.reshape(4, 128, 2, 128).transpose(1, 0, 2, 3) \
        .reshape(128, -1).astype(np.float32).copy()
    out["b8"] = b8.reshape(2, 128).T.astype(np.float32).copy()
    out["w9t"] = f64(inp["W9"]).T.reshape(2, 128, 63).transpose(1, 0, 2) \
        .reshape(128, 2 * 63).astype(np.float32).copy()
    out["b9"] = f64(inp["b9"])[:, None].astype(np.float32)
    return out


def fold_sample(sample_x, h, split=2):
    """sample_x: (3, N) float32. Returns per-core arrays for half h."""
    x = np.asarray(sample_x, np.float64)
    xx = (x * x).sum(0)
    N = x.shape[1]
    NH = N // split
    xo = x[:, h * NH:(h + 1) * NH]
    return {
        "xr": np.concatenate([x, -0.5 * xx[None, :]], 0).astype(np.float32),
        "xa": np.concatenate([xo, np.ones((1, NH))], 0).astype(np.float32),
    }


def make_in_maps(inputs, split=2):
    """inputs: reference setup_inputs() dict (numpy). split cores/sample."""
    shared = fold_weights(inputs)
    x = np.asarray(inputs["x"])
    B = x.shape[0]
    in_maps = []
    for c in range(B * split):
        b, h = c // split, c % split
        m = dict(shared)
        m.update(fold_sample(x[b], h, split))
        in_maps.append(m)
    return in_maps


def cast_inputs(in_maps, nc):
    dts = {}
    for alloc in nc.m.functions[0].allocations:
        if isinstance(alloc, mybir.MemoryLocationSet) and alloc.kind == "ExternalInput":
            dts[alloc.memorylocations[0].name] = mybir.dt.np(alloc.dtype)
    outs = []
    for m in in_maps:
        outs.append({k: np.ascontiguousarray(np.asarray(v).astype(dts[k]))
                     for k, v in m.items() if k in dts})
    return outs


# =====================================================================
# Harness entry point
# =====================================================================
_CACHE = {}


def _make_runner(nc, n_cores):
    """Compile-once SPMD runner (mirrors bass2jax.run_bass_via_pjrt but
    caches the jitted executable across kernel() calls)."""
    import jax
    from concourse import bass2jax
    from concourse.bass2jax import _bass_exec_p, partition_id_tensor, \
        install_neuronx_cc_hook

    install_neuronx_cc_hook()
    partition_name = nc.partition_id_tensor.name if nc.partition_id_tensor else None
    in_names, out_names, out_avals, zero_shapes = [], [], [], []
    for alloc in nc.m.functions[0].allocations:
        if not isinstance(alloc, mybir.MemoryLocationSet):
            continue
        name = alloc.memorylocations[0].name
        if alloc.kind == "ExternalInput":
            if name != partition_name:
                in_names.append(name)
        elif alloc.kind == "ExternalOutput":
            shape = tuple(alloc.tensor_shape)
            dtype = mybir.dt.np(alloc.dtype)
            out_names.append(name)
            out_avals.append(jax.core.ShapedArray(shape, dtype))
            zero_shapes.append((shape, dtype))
    n_params = len(in_names)
    all_names = in_names + out_names + ([partition_name] if partition_name else [])
    donate = tuple(range(n_params, n_params + len(out_names)))

    def _body(*args):
        operands = list(args)
        if partition_name is not None:
            operands.append(partition_id_tensor())
        return tuple(_bass_exec_p.bind(
            *operands, out_avals=tuple(out_avals), in_names=tuple(all_names),
            out_names=tuple(out_names), lowering_input_output_aliases=(),
            sim_require_finite=True, sim_require_nnan=True, nc=nc))

    from jax.experimental.shard_map import shard_map
    from jax.sharding import Mesh, PartitionSpec
    mesh = Mesh(np.asarray(jax.devices()[:n_cores]), ("core",))
    in_specs = (PartitionSpec("core"),) * (n_params + len(out_names))
    out_specs = (PartitionSpec("core"),) * len(out_names)
    jf = jax.jit(
        shard_map(_body, mesh=mesh, in_specs=in_specs, out_specs=out_specs,
                  check_rep=False),
        donate_argnums=donate, keep_unused=True)

    import hashlib
    dev_cache = {}

    def _zeros_dev():
        return [jax.numpy.zeros((n_cores * shape[0],) + shape[1:], dtype)
                for shape, dtype in zero_shapes]

    def run(in_maps):
        h = hashlib.md5()
        for name in in_names:
            for m in in_maps:
                h.update(np.asarray(m[name]).tobytes())
        key = h.hexdigest()
        if key not in dev_cache:
            dev_cache.clear()
            arrs = [np.concatenate([np.asarray(m[name]) for m in in_maps], axis=0)
                    for name in in_names]
            dev_cache[key] = [jax.device_put(a) for a in arrs]
        args = list(dev_cache[key]) + _zeros_dev()
        outs = jf(*args)
        return [{n: np.asarray(outs[i]).reshape((n_cores,) + zero_shapes[i][0])[c]
                 for i, n in enumerate(out_names)}
                for c in range(n_cores)]

    return run


def kernel(**inputs):
    """DGCNN forward. inputs keyed as reference.setup_inputs(); returns
    (B, 63, N) float32. Two NeuronCores per sample (split halves)."""
    from concourse.bass_utils import run_bass_kernel_spmd

    x = np.asarray(inputs["x"])
    B, _, N = x.shape
    SPLIT = 2
    n_cores = B * SPLIT
    key = (B, N)
    if key not in _CACHE:
        nc = build_core(N=N, split=SPLIT)
        runner = None
        try:
            runner = _make_runner(nc, n_cores)
        except Exception:
            runner = None
        _CACHE[key] = (nc, runner)
    nc, runner = _CACHE[key]
    in_maps = cast_inputs(make_in_maps(inputs, split=SPLIT), nc)
    NH = N // SPLIT

    def assemble(res_list):
        full = np.zeros((B, 63, N), np.float32)
        for c in range(n_cores):
            b, h = c // SPLIT, c % SPLIT
            full[b][:, h * NH:(h + 1) * NH] = res_list[c]["out"]
        return full

    if runner is not None:
        try:
            return assemble(runner(in_maps))
        except Exception:
            _CACHE[key] = (nc, None)
    res = run_bass_kernel_spmd(nc, in_maps, core_ids=list(range(n_cores)))
    return assemble(res.results)


# revision 43
# speedup vs baseline: 1.0281x; 1.0281x over previous
"""DGCNN Bass kernel for trn2 — 8-core pair-split (2 cores per sample).

Per core (half a sample: NH=2048 of N=4096 points, k=40 neighbors):
  1. kNN scores for own queries vs ALL points via K=4 fp32r matmuls.
  2. top-40: window-max pooling (8-wide) -> top-40 windows via DVE
     max/max_index rounds -> gather 40 windows (d=8) -> top-40 elements.
  3. EdgeConv rounds with gather-after-matmul factorization:
     conv([nbr-ctr, ctr]) = A[:, j] + B[:, i]; A tables over ALL points
     (rebuilt from pair-AllGathered x1/x2), B tables over own points.
  4. Global-max head with W7 split; per-pair AllGather for the global max.

BN scales folded into weights on host; LeakyReLU on the Act engine.
"""
import numpy as np
import concourse.bass as bass
import concourse.mybir as mybir
from concourse.bacc import Bacc
from concourse.tile import TileContext

F32 = mybir.dt.float32
F32R = mybir.dt.float32r
F16 = mybir.dt.float16
BF16 = mybir.dt.bfloat16
U16 = mybir.dt.uint16
I16 = mybir.dt.int16
I32 = mybir.dt.int32
AX = mybir.AxisListType
OP = mybir.AluOpType
ACTF = mybir.ActivationFunctionType

KNBR = 40
NEG = -1e30
LEAK = 0.2


def build_core(N=4096, split=2):
    """One core's program. split=2: core owns NH=N//2 points (which half is
    decided purely by the host-fed data); pair collectives exchange x1/x2/g."""
    nc = Bacc(None)
    NH = N // split
    T = NH // 128          # own tiles
    PAIRS = T // 2
    RG = [[2 * i, 2 * i + 1] for i in range(4)] if split == 2 else None

    def din(name, shape, dt=F32):
        return nc.dram_tensor(name, shape, dt, kind="ExternalInput")

    xr_d = din("xr", [4, N], F32R)
    xa_d = din("xa", [4, NH], F32R)
    a1w_d = din("a1w", [3, 64], F32R)
    b1w_d = din("b1w", [3, 64], F32R)
    b1_d = din("b1", [64, 1])
    w2t_d = din("w2t", [64, 64], BF16)
    b2_d = din("b2", [64, 1])
    a3w_d = din("a3w", [64, 64], BF16)
    b3w_d = din("b3w", [64, 64], BF16)
    b3_d = din("b3", [64, 1])
    w4t_d = din("w4t", [64, 64], BF16)
    b4_d = din("b4", [64, 1])
    a5w_d = din("a5w", [64, 64], BF16)
    b5w_d = din("b5w", [64, 64], BF16)
    b5_d = din("b5", [64, 1])
    w6t_d = din("w6t", [64, 3 * 1024], BF16)
    b6_d = din("b6", [128, 8])
    w7gt_d = din("w7gt", [128, 8 * 4 * 128])
    b7_d = din("b7", [128, 4])
    w7xt_d = din("w7xt", [64, 3 * 4 * 128], BF16)
    w8t_d = din("w8t", [128, 4 * 2 * 128], BF16)
    b8_d = din("b8", [128, 2])
    w9t_d = din("w9t", [128, 2 * 63], BF16)
    b9_d = din("b9", [63, 1])

    out_d = nc.dram_tensor("out", [63, NH], F32, kind="ExternalOutput")
    debug = bool(int(__import__("os").environ.get("DGCNN_DEBUG", "0")))
    if debug:
        idx_dbg = nc.dram_tensor("idx_dbg", [128, T * KNBR], U16,
                                 kind="ExternalOutput")
        x1_dbg = nc.dram_tensor("x1_dbg", [64, NH], F32, kind="ExternalOutput")
        x2_dbg = nc.dram_tensor("x2_dbg", [64, NH], F32, kind="ExternalOutput")
        x3_dbg = nc.dram_tensor("x3_dbg", [64, NH], F32, kind="ExternalOutput")
        g_dbg = nc.dram_tensor("g_dbg", [128, 8], F32, kind="ExternalOutput")
        xf_dbg = nc.dram_tensor("xf_dbg", [64, N], F32, kind="ExternalOutput")

    def mm_r(ps, lhsT, rhs, **kw):
        nc.tensor.matmul(ps, lhsT, rhs, **kw)

    with TileContext(nc) as tc:
        with tc.tile_pool(name="persist", bufs=1) as pp:
            x1o = pp.tile([64, NH], BF16)
            x2o = pp.tile([64, NH], BF16)
            x3o = pp.tile([64, NH], BF16)
            xf = pp.tile([64, N], BF16)        # gathered full x1 / x2 (reused)
            arep = pp.tile([128, N], F32)      # A table (dup on both halves)
            brep = pp.tile([128, NH], F32)     # B table (shift-dup)
            wraps = pp.tile([128, 320 * T], U16)
            idx128 = pp.tile([128, 320 * PAIRS], U16)
            nbr_all = pp.tile([128, T * KNBR], U16)
            iota8 = pp.tile([128, 8], U16)
            nc.gpsimd.iota(iota8[:], pattern=[[1, 8]], base=0,
                           channel_multiplier=0)
            iota8m7 = pp.tile([128, 8], I16)
            nc.vector.tensor_scalar(iota8m7[:], iota8[:], 7, None,
                                    op0=OP.subtract)
            iotaJ = pp.tile([128, N], U16)
            nc.gpsimd.iota(iotaJ[:], pattern=[[1, N]], base=0,
                           channel_multiplier=0)
            iotaR1 = pp.tile([128, KNBR], U16)
            nc.gpsimd.iota(iotaR1[:], pattern=[[1, KNBR]], base=1,
                           channel_multiplier=0)
            zc = pp.tile([128, 1], F32)
            nc.gpsimd.memset(zc[:], 0.0)

            # =====================================================
            # EdgeConv machinery
            # =====================================================
            def build_a_table(aw_t, src_full, kdim, fp32=False, order=None):
                """A table over ALL N points -> arep (f32, duplicated halves).
                order: chunk emission order (readiness after split AllGathers)."""
                with tc.tile_pool(name="ta_ps", bufs=2, space="PSUM") as tps:
                    for j in (order or range(N // 512)):
                        psa = tps.tile([64, 512], F32, tag="psa")
                        if fp32:
                            mm_r(psa[:], aw_t[:], src_full[:, j * 512:(j + 1) * 512],
                                 start=True, stop=True)
                        else:
                            nc.tensor.matmul(psa[:], aw_t[:],
                                             src_full[:, j * 512:(j + 1) * 512],
                                             start=True, stop=True)
                        nc.scalar.copy(out=arep[0:64, j * 512:(j + 1) * 512],
                                       in_=psa[:])
                        nc.sync.dma_start(out=arep[64:128, j * 512:(j + 1) * 512],
                                          in_=arep[0:64, j * 512:(j + 1) * 512])

            def build_b_table(bw_t, bias_t, src_own, fp32=False):
                """B table (+bias) over OWN points -> brep (f32, shift-dup)."""
                with tc.tile_pool(name="tb_ps", bufs=2, space="PSUM") as tps:
                    for j in range(NH // 512):
                        psb = tps.tile([64, 512], F32, tag="psb")
                        if fp32:
                            mm_r(psb[:], bw_t[:], src_own[:, j * 512:(j + 1) * 512],
                                 start=True, stop=True)
                        else:
                            nc.tensor.matmul(psb[:], bw_t[:],
                                             src_own[:, j * 512:(j + 1) * 512],
                                             start=True, stop=True)
                        nc.scalar.activation(brep[0:64, j * 512:(j + 1) * 512],
                                             psb[:], ACTF.Identity, bias=bias_t[:])
                    nc.sync.dma_start(out=brep[64:128, 0:NH - 128],
                                      in_=brep[0:64, 128:NH])

            def edge_round(wt_t, bias_t, xout, last=False, half_hook=None):
                with tc.tile_pool(name="er_sb", bufs=2) as esp, \
                     tc.tile_pool(name="er_ps", bufs=2, space="PSUM") as eps:
                    for p in range(PAIRS):
                        if half_hook is not None and p > 0 and p % (PAIRS // 4) == 0:
                            half_hook(p // (PAIRS // 4) - 1)
                        tA = 2 * p
                        ga = esp.tile([128, KNBR * 128], F32, tag="ga")
                        nc.gpsimd.ap_gather(
                            ga[:], arep[:],
                            idx128[:, p * 320:(p + 1) * 320].bitcast(I16),
                            channels=128, num_elems=N, d=1, num_idxs=KNBR * 128)
                        if last:
                            mx = esp.tile([128, 128], F32, tag="mx")
                            nc.vector.tensor_reduce(
                                out=mx[:], in_=ga.rearrange("p (k i) -> p i k", k=KNBR),
                                axis=AX.X, op=OP.max)
                            zz = esp.tile([128, 128], F32, tag="zz")
                            nc.vector.tensor_tensor(
                                out=zz[:], in0=mx[:],
                                in1=brep[:, tA * 128: tA * 128 + 128], op=OP.add)
                            xo = esp.tile([128, 128], BF16, tag="xo")
                            nc.vector.scalar_tensor_tensor(
                                out=xo[:], in0=zz[:], scalar=LEAK, in1=zz[:],
                                op0=OP.mult, op1=OP.max)
                        else:
                            bview = brep[:, tA * 128: tA * 128 + 128] \
                                .unsqueeze(1).broadcast_to([128, KNBR, 128])
                            e = esp.tile([128, KNBR * 128], BF16, tag="e")
                            nc.vector.tensor_tensor(
                                out=e.rearrange("p (k i) -> p k i", k=KNBR),
                                in0=ga.rearrange("p (k i) -> p k i", k=KNBR),
                                in1=bview, op=OP.add)
                            nc.vector.scalar_tensor_tensor(
                                out=e[:], in0=e[:], scalar=LEAK, in1=e[:],
                                op0=OP.mult, op1=OP.max)
                            # conv2 in 2048-col PSUM chunks (4 banks each);
                            # per-chunk k=16 max-reduce on DVE
                            NCH = 3
                            widths = [2048, 2048, 1024]
                            pmax = esp.tile([128, NCH * 128], F32, tag="pmax")
                            col = 0
                            for c, wdt in enumerate(widths):
                                cps_t = eps.tile([128, 2048], F32, tag="cps")
                                cps = cps_t[:, 0:wdt]
                                for j in range(wdt // 512):
                                    for hh in range(2):
                                        pb = 64 * hh
                                        nc.tensor.matmul(
                                            cps[pb:pb + 64, j * 512:(j + 1) * 512],
                                            wt_t[pb:pb + 64, :],
                                            e[pb:pb + 64, col + j * 512:
                                              col + (j + 1) * 512],
                                            start=True, stop=True)
                                nc.vector.tensor_reduce(
                                    out=pmax[:, c * 128:(c + 1) * 128],
                                    in_=cps.rearrange("p (k i) -> p i k", k=wdt // 128),
                                    axis=AX.X, op=OP.max)
                                col += wdt
                            mx = esp.tile([128, 128], F32, tag="mx")
                            nc.vector.tensor_reduce(
                                out=mx[:], in_=pmax.rearrange("p (k i) -> p i k", k=NCH),
                                axis=AX.X, op=OP.max)
                            xz = esp.tile([128, 128], F32, tag="xz")
                            nc.scalar.activation(xz[:], mx[:], ACTF.Identity,
                                                 bias=bias_t[:])
                            xo = esp.tile([128, 128], BF16, tag="xo")
                            nc.vector.scalar_tensor_tensor(
                                out=xo[:], in0=xz[:], scalar=LEAK, in1=xz[:],
                                op0=OP.mult, op1=OP.max)
                        nc.sync.dma_start(out=xout[:, tA * 128:(tA + 1) * 128],
                                          in_=xo[0:64, :])
                        nc.sync.dma_start(out=xout[:, (tA + 1) * 128:(tA + 2) * 128],
                                          in_=xo[64:128, :])
                    if half_hook is not None:
                        half_hook(3)

            def load_bias128(bias_d_, pool, tag):
                bt = pool.tile([128, 1], F32, tag=tag)
                nc.sync.dma_start(out=bt[0:64, :], in_=bias_d_[:])
                nc.sync.dma_start(out=bt[64:128, :], in_=bias_d_[:])
                return bt

            def allgather_half(xown, hf, dram_pool, nm):
                """Pair-AllGather column-quarter hf of own (64, NH) bf16 into
                xf. Emitted mid-round so the collective overlaps later pairs."""
                HQ = NH // 4
                c0 = hf * HQ
                if split == 1:
                    nc.sync.dma_start(out=xf[:, c0:c0 + HQ],
                                      in_=xown[:, c0:c0 + HQ])
                    return
                gin = dram_pool.tile([64, HQ], BF16, name=f"gin_{nm}{hf}")
                gout = dram_pool.tile([128, HQ], BF16, name=f"gout_{nm}{hf}")
                nc.sync.dma_start(out=gin[:], in_=xown[:, c0:c0 + HQ])
                nc.gpsimd.collective_compute(
                    "AllGather", OP.bypass, replica_groups=RG,
                    ins=[gin[:]], outs=[gout[:]])
                nc.sync.dma_start(out=xf[:, c0:c0 + HQ], in_=gout[0:64, :])
                nc.sync.dma_start(out=xf[:, NH + c0:NH + c0 + HQ],
                                  in_=gout[64:128, :])

            # =====================================================
            # Stage A: kNN + top-40 per tile
            # =====================================================
            xr = pp.tile([4, N], F32R, name="xr")
            nc.sync.dma_start(out=xr[:], in_=xr_d[:])
            xa = pp.tile([4, NH], F32R, name="xa")
            nc.sync.dma_start(out=xa[:], in_=xa_d[:])
            wts_ctx = tc.tile_pool(name="wts", bufs=1)
            wp = wts_ctx.__enter__()
            dram_ctx = tc.tile_pool(name="dram", bufs=1, space="DRAM")
            dram = dram_ctx.__enter__()
            a1w = wp.tile([3, 64], F32R)
            nc.sync.dma_start(out=a1w[:], in_=a1w_d[:])
            b1w = wp.tile([3, 64], F32R)
            nc.sync.dma_start(out=b1w[:], in_=b1w_d[:])
            b1t = wp.tile([64, 1], F32)
            nc.sync.dma_start(out=b1t[:], in_=b1_d[:])
            # EC1 tables (from coordinates) run on PE/Act during selection
            build_a_table(a1w, xr[0:3, :], 3, fp32=True)
            build_b_table(b1w, b1t, xa[0:3, :], fp32=True)
            with tc.tile_pool(name="sel_sb", bufs=2) as sp, \
                 tc.tile_pool(name="sel_ps", bufs=2, space="PSUM") as sps:
                for t in range(T):
                    s_sb = sp.tile([128, N], F32, tag="s_sb")
                    pooled = sp.tile([128, N // 8], F32, tag="pooled", bufs=3)
                    for h in range(2):
                        ps = sps.tile([128, N // 2], F32, tag="score")
                        for j in range(N // 2 // 512):
                            col = h * (N // 2) + j * 512
                            mm_r(ps[:, j * 512:(j + 1) * 512],
                                 xa[:, t * 128:(t + 1) * 128],
                                 xr[:, col:col + 512],
                                 start=True, stop=True)
                        nc.scalar.copy(out=s_sb[:, h * (N // 2):(h + 1) * (N // 2)],
                                       in_=ps[:])
                        # 8-wide window max (DVE; Pool has no ALU path)
                        W = N // 16          # windows per half
                        nc.vector.tensor_reduce(
                            out=pooled[:, h * W:(h + 1) * W],
                            in_=ps.rearrange("p (w k) -> p w k", k=8),
                            axis=AX.X, op=OP.max)
                    # tau = 40th-largest window max; the match_replace NEG
                    # marks left in `pooled` identify the top-40 windows
                    NW = N // 8
                    CAP = 322        # exactly 40 windows * 8 slots + dump
                    m8 = sp.tile([128, 8], F32, tag="m8", bufs=3)
                    for r in range(5):
                        nc.vector.max(out=m8[:], in_=pooled[:])
                        nc.vector.match_replace(out=pooled[:], in_to_replace=m8[:],
                                                in_values=pooled[:], imm_value=NEG)
                    tau = m8[:, 7:8]
                    wmask = sp.tile([128, NW], BF16, tag="wmask", bufs=3)
                    nc.vector.tensor_scalar(wmask[:], pooled[:], -1e29, None,
                                            op0=OP.is_le)
                    wrank = sp.tile([128, NW], F32, tag="wrank", bufs=3)
                    nc.vector.tensor_tensor_scan(
                        wrank[:], wmask[:], zc.broadcast_to([128, NW]), 0.0,
                        op0=OP.add, op1=OP.add)
                    wslot8 = sp.tile([128, NW], I16, tag="wslot8", bufs=3)
                    nc.vector.scalar_tensor_tensor(
                        out=wslot8[:], in0=wrank[:], scalar=8.0, in1=wmask[:],
                        op0=OP.mult, op1=OP.mult)
                    # element slots: eslot[w*8+j] = wslot8[w] + j - 7
                    # (non-candidate windows give j-7 <= 0 -> dropped)
                    eslot = sp.tile([128, N], I16, tag="eslot")
                    nc.vector.tensor_tensor(
                        out=eslot.rearrange("p (w j) -> p w j", j=8),
                        in0=wslot8.unsqueeze(2).broadcast_to([128, NW, 8]),
                        in1=iota8m7.unsqueeze(1).broadcast_to([128, NW, 8]),
                        op=OP.add)
                    # shifted scores: boundary at 0 keeps f16 ordering exact
                    sb1 = sp.tile([128, N], F16, tag="sb1")
                    negtau1 = sp.tile([128, 1], F32, tag="negtau1", bufs=3)
                    nc.vector.tensor_scalar(negtau1[:], tau, -1.0, None,
                                            op0=OP.mult)
                    nc.scalar.activation(sb1[:], s_sb[:], ACTF.Identity,
                                         bias=negtau1[:])
                    candh = sp.tile([128, CAP], F16, tag="candh", bufs=3)
                    nc.gpsimd.local_scatter(candh[:], sb1[:], eslot[:],
                                            channels=128, num_elems=CAP,
                                            num_idxs=N)
                    candidx = sp.tile([128, CAP], U16, tag="candidx", bufs=3)
                    nc.gpsimd.local_scatter(candidx[:], iotaJ[:], eslot[:],
                                            channels=128, num_elems=CAP,
                                            num_idxs=N)
                    # exactly 40 windows are marked, so slots 1..320 are all
                    # filled -- no empty-slot masking needed
                    candf = sp.tile([128, CAP], F32, tag="candf", bufs=3)
                    nc.vector.tensor_scalar(candf[:], candh[:], 0.0, None,
                                            op0=OP.add)
                    pos40 = sp.tile([128, KNBR], U16, tag="pos40", bufs=3)
                    cv = candf[:, 1:321]
                    for r in range(5):
                        nc.vector.max(out=m8[:], in_=cv)
                        nc.vector.max_index(out=pos40[:, r * 8:(r + 1) * 8],
                                            in_max=m8[:], in_values=cv)
                        nc.vector.match_replace(out=cv, in_to_replace=m8[:],
                                                in_values=cv, imm_value=NEG)
                    # map candidate positions back to global element ids
                    posi = sp.tile([128, KNBR], I16, tag="posi", bufs=3)
                    nc.vector.tensor_scalar(posi[:], pos40[:], 1, None, op0=OP.add)
                    winv = sp.tile([128, CAP], U16, tag="winv", bufs=3)
                    nc.gpsimd.local_scatter(winv[:], iotaR1[:], posi[:],
                                            channels=128, num_elems=CAP,
                                            num_idxs=KNBR)
                    winm1 = sp.tile([128, CAP], I16, tag="winm1", bufs=3)
                    nc.vector.tensor_scalar(winm1[:], winv[:], 1, None,
                                            op0=OP.subtract)
                    nc.gpsimd.local_scatter(
                        nbr_all[:, t * KNBR:(t + 1) * KNBR], candidx[:], winm1[:],
                        channels=128, num_elems=KNBR, num_idxs=CAP)
                    # wrapped list: wrap[ilo, kk*8+ihi] = nbr[ihi*16+ilo, kk]
                    pbase = 64 * (t % 2)
                    for ihi in range(8):
                        dst = wraps[pbase:pbase + 16, t * 320:(t + 1) * 320] \
                            .rearrange("p (k e) -> p k e", e=8)[:, :, ihi:ihi + 1]
                        nc.sync.dma_start(
                            out=dst,
                            in_=nbr_all[ihi * 16:(ihi + 1) * 16,
                                        t * KNBR:(t + 1) * KNBR].unsqueeze(2))
                    # replicate to the second quadrant (+32 partitions)
                    nc.sync.dma_start(
                        out=wraps[pbase + 32: pbase + 48, t * 320:(t + 1) * 320],
                        in_=wraps[pbase: pbase + 16, t * 320:(t + 1) * 320])
                    if t % 2 == 1:
                        p = t // 2
                        rep16 = [i % 16 for i in range(32)]
                        nc.vector.stream_shuffle(
                            idx128[0:64, p * 320:(p + 1) * 320],
                            wraps[0:64, (2 * p) * 320:(2 * p + 1) * 320],
                            mask=rep16)
                        nc.vector.stream_shuffle(
                            idx128[64:128, p * 320:(p + 1) * 320],
                            wraps[64:128, (2 * p + 1) * 320:(2 * p + 2) * 320],
                            mask=rep16)

            if True:
                # ---- EdgeConv 1 ----
                w2t = wp.tile([128, 64], BF16)
                nc.sync.dma_start(out=w2t[0:64, :], in_=w2t_d[:])
                nc.sync.dma_start(out=w2t[64:128, :], in_=w2t_d[:])
                b2r = load_bias128(b2_d, wp, "b2r")
                # weight loads for later rounds (overlap with EC1 compute)
                a3w = wp.tile([64, 64], BF16)
                nc.sync.dma_start(out=a3w[:], in_=a3w_d[:])
                b3w = wp.tile([64, 64], BF16)
                nc.sync.dma_start(out=b3w[:], in_=b3w_d[:])
                b3t = wp.tile([64, 1], F32)
                nc.sync.dma_start(out=b3t[:], in_=b3_d[:])
                w4t = wp.tile([128, 64], BF16)
                nc.sync.dma_start(out=w4t[0:64, :], in_=w4t_d[:])
                nc.sync.dma_start(out=w4t[64:128, :], in_=w4t_d[:])
                b4r = load_bias128(b4_d, wp, "b4r")
                a5w = wp.tile([64, 64], BF16)
                nc.sync.dma_start(out=a5w[:], in_=a5w_d[:])
                b5w = wp.tile([64, 64], BF16)
                nc.sync.dma_start(out=b5w[:], in_=b5w_d[:])
                b5t = wp.tile([64, 1], F32)
                nc.sync.dma_start(out=b5t[:], in_=b5_d[:])

                READY = [0, 4, 1, 5, 2, 6, 3, 7]   # xf chunks by AG-quarter
                edge_round(w2t, b2r, x1o,
                           half_hook=lambda h: allgather_half(x1o, h, dram, "x1"))
                build_b_table(b3w, b3t, x1o)
                build_a_table(a3w, xf, 64, order=READY)
                edge_round(w4t, b4r, x2o,
                           half_hook=lambda h: allgather_half(x2o, h, dram, "x2"))
                build_b_table(b5w, b5t, x2o)
                build_a_table(a5w, xf, 64, order=READY)
                edge_round(None, None, x3o, last=True)

                # =====================================================
                # Head
                # =====================================================
                with tc.tile_pool(name="hd", bufs=1) as hp, \
                     tc.tile_pool(name="hd_sb", bufs=3) as hsp, \
                     tc.tile_pool(name="hd_ps", bufs=2, space="PSUM") as hps:
                    w6t = hp.tile([64, 3 * 1024], BF16)
                    nc.sync.dma_start(out=w6t[:], in_=w6t_d[:])
                    b6t = hp.tile([128, 8], F32)
                    nc.sync.dma_start(out=b6t[:], in_=b6_d[:])
                    w7gt = hp.tile([128, 8 * 4 * 128], F32)
                    nc.sync.dma_start(out=w7gt[:], in_=w7gt_d[:])
                    b7t = hp.tile([128, 4], F32)
                    nc.sync.dma_start(out=b7t[:], in_=b7_d[:])
                    w7xt = hp.tile([64, 3 * 4 * 128], BF16)
                    nc.sync.dma_start(out=w7xt[:], in_=w7xt_d[:])
                    w8t = hp.tile([128, 4 * 2 * 128], BF16)
                    nc.sync.dma_start(out=w8t[:], in_=w8t_d[:])
                    b8t = hp.tile([128, 2], F32)
                    nc.sync.dma_start(out=b8t[:], in_=b8_d[:])
                    w9t = hp.tile([128, 2 * 63], BF16)
                    nc.sync.dma_start(out=w9t[:], in_=w9t_d[:])
                    b9t = hp.tile([63, 1], F32)
                    nc.sync.dma_start(out=b9t[:], in_=b9_d[:])

                    if debug:
                        nc.sync.dma_start(out=idx_dbg[:], in_=nbr_all[:])
                        for src, dst in [(x1o, x1_dbg), (x2o, x2_dbg),
                                         (x3o, x3_dbg)]:
                            db = hp.tile([64, NH], F32, name=f"db_{dst.name}")
                            nc.vector.tensor_scalar(db[:], src[:], 0.0, None,
                                                    op0=OP.add)
                            nc.sync.dma_start(out=dst[:], in_=db[:])
                        dbf = hp.tile([64, N], F32, name="dbf")
                        nc.vector.tensor_scalar(dbf[:], xf[:], 0.0, None,
                                                op0=OP.add)
                        nc.sync.dma_start(out=xf_dbg[:], in_=dbf[:])
                    xs_ = [x1o, x2o, x3o]
                    NC6 = NH // 512
                    gtmp = hp.tile([128, 8 * NC6], F32)
                    for o in range(8):
                        for n in range(NC6):
                            ps6 = hps.tile([128, 512], F32, tag="ps6")
                            for kp in range(3):
                                nc.tensor.matmul(
                                    ps6[:],
                                    w6t[:, kp * 1024 + o * 128: kp * 1024 + (o + 1) * 128],
                                    xs_[kp][:, n * 512:(n + 1) * 512],
                                    start=(kp == 0), stop=(kp == 2))
                            nc.vector.tensor_reduce(
                                out=gtmp[:, o * NC6 + n: o * NC6 + n + 1],
                                in_=ps6[:], axis=AX.X, op=OP.max)
                    gpart = hp.tile([128, 8], F32)
                    nc.vector.tensor_reduce(
                        out=gpart[:], in_=gtmp.rearrange("p (o n) -> p o n", o=8),
                        axis=AX.X, op=OP.max)
                    g = hp.tile([128, 8], F32)
                    if split == 2:
                        ggin = dram.tile([128, 8], F32, name="ggin")
                        ggout = dram.tile([256, 8], F32, name="ggout")
                        nc.sync.dma_start(out=ggin[:], in_=gpart[:])
                        nc.gpsimd.collective_compute(
                            "AllGather", OP.bypass, replica_groups=RG,
                            ins=[ggin[:]], outs=[ggout[:]])
                        gA = hp.tile([128, 8], F32)
                        nc.sync.dma_start(out=gA[:], in_=ggout[0:128, :])
                        gB = hp.tile([128, 8], F32)
                        nc.sync.dma_start(out=gB[:], in_=ggout[128:256, :])
                        nc.vector.tensor_tensor(out=g[:], in0=gA[:], in1=gB[:],
                                                op=OP.max)
                    else:
                        nc.vector.tensor_tensor(out=g[:], in0=gpart[:],
                                                in1=gpart[:], op=OP.max)
                    nc.vector.tensor_tensor(out=g[:], in0=g[:], in1=b6t[:], op=OP.add)
                    g2 = hp.tile([128, 8], F32)
                    nc.vector.scalar_tensor_tensor(
                        out=g2[:], in0=g[:], scalar=LEAK, in1=g[:],
                        op0=OP.mult, op1=OP.max)
                    if debug:
                        nc.sync.dma_start(out=g_dbg[:], in_=g2[:])

                    ps7v = hps.tile([128, 4], F32, tag="ps7v", bufs=1)
                    for m in range(4):
                        for o in range(8):
                            nc.tensor.matmul(
                                ps7v[:, m:m + 1],
                                w7gt[:, (o * 4 + m) * 128:(o * 4 + m + 1) * 128],
                                g2[:, o:o + 1], start=(o == 0), stop=(o == 7))
                    v7 = hp.tile([128, 4], F32)
                    nc.vector.tensor_tensor(out=v7[:], in0=ps7v[:], in1=b7t[:],
                                            op=OP.add)

                    for n in range(NC6):
                        y7 = hsp.tile([128, 4 * 512], BF16, tag="y7")
                        for m in range(4):
                            ps7 = hps.tile([128, 512], F32, tag="ps7")
                            for kp in range(3):
                                nc.tensor.matmul(
                                    ps7[:],
                                    w7xt[:, (kp * 4 + m) * 128:(kp * 4 + m + 1) * 128],
                                    xs_[kp][:, n * 512:(n + 1) * 512],
                                    start=(kp == 0), stop=(kp == 2))
                            t7 = hsp.tile([128, 512], F32, tag="t7")
                            nc.scalar.activation(t7[:], ps7[:], ACTF.Identity,
                                                 bias=v7[:, m:m + 1])
                            nc.vector.scalar_tensor_tensor(
                                out=y7[:, m * 512:(m + 1) * 512], in0=t7[:],
                                scalar=LEAK, in1=t7[:], op0=OP.mult, op1=OP.max)
                        y8 = hsp.tile([128, 2 * 512], BF16, tag="y8")
                        for m in range(2):
                            ps8 = hps.tile([128, 512], F32, tag="ps8")
                            for k in range(4):
                                nc.tensor.matmul(
                                    ps8[:], w8t[:, (k * 2 + m) * 128:(k * 2 + m + 1) * 128],
                                    y7[:, k * 512:(k + 1) * 512],
                                    start=(k == 0), stop=(k == 3))
                            t8 = hsp.tile([128, 512], F32, tag="t7")
                            nc.scalar.activation(t8[:], ps8[:], ACTF.Identity,
                                                 bias=b8t[:, m:m + 1])
                            nc.vector.scalar_tensor_tensor(
                                out=y8[:, m * 512:(m + 1) * 512], in0=t8[:],
                                scalar=LEAK, in1=t8[:], op0=OP.mult, op1=OP.max)
                        ps9 = hps.tile([63, 512], F32, tag="ps9", bufs=1)
                        for k in range(2):
                            nc.tensor.matmul(ps9[:], w9t[:, k * 63:(k + 1) * 63],
                                             y8[:, k * 512:(k + 1) * 512],
                                             start=(k == 0), stop=(k == 1))
                        o9 = hsp.tile([63, 512], F32, tag="o9")
                        nc.scalar.activation(o9[:], ps9[:], ACTF.Identity,
                                             bias=b9t[:])
                        nc.sync.dma_start(out=out_d[:, n * 512:(n + 1) * 512],
                                          in_=o9[:])
                dram_ctx.__exit__(None, None, None)
                wts_ctx.__exit__(None, None, None)

    nc.finalize()
    return nc


# =====================================================================
# Host-side folding
# =====================================================================
def fold_weights(inp):
    """inp: the reference setup_inputs() dict. Returns dict of shared
    (sample-independent) device arrays."""
    def f64(a):
        return np.asarray(a, np.float64)

    out = {}
    W1, s1, b1 = f64(inp["W1"]), f64(inp["s1"]), f64(inp["b1"])
    W1a, W1b = W1[:, :3], W1[:, 3:]
    out["a1w"] = (s1[:, None] * W1a).T.astype(np.float32).copy()
    out["b1w"] = (s1[:, None] * (W1b - W1a)).T.astype(np.float32).copy()
    out["b1"] = b1[:, None].astype(np.float32)
    W2, s2, b2 = f64(inp["W2"]), f64(inp["s2"]), f64(inp["b2"])
    assert (s2 > 0).all()
    out["w2t"] = (s2[:, None] * W2).T.astype(np.float32).copy()
    out["b2"] = b2[:, None].astype(np.float32)
    W3, s3, b3 = f64(inp["W3"]), f64(inp["s3"]), f64(inp["b3"])
    W3a, W3b = W3[:, :64], W3[:, 64:]
    out["a3w"] = (s3[:, None] * W3a).T.astype(np.float32).copy()
    out["b3w"] = (s3[:, None] * (W3b - W3a)).T.astype(np.float32).copy()
    out["b3"] = b3[:, None].astype(np.float32)
    W4, s4, b4 = f64(inp["W4"]), f64(inp["s4"]), f64(inp["b4"])
    assert (s4 > 0).all()
    out["w4t"] = (s4[:, None] * W4).T.astype(np.float32).copy()
    out["b4"] = b4[:, None].astype(np.float32)
    W5, s5, b5 = f64(inp["W5"]), f64(inp["s5"]), f64(inp["b5"])
    W5a, W5b = W5[:, :64], W5[:, 64:]
    out["a5w"] = (s5[:, None] * W5a).T.astype(np.float32).copy()
    out["b5w"] = (s5[:, None] * (W5b - W5a)).T.astype(np.float32).copy()
    out["b5"] = b5[:, None].astype(np.float32)
    W6, s6, b6 = f64(inp["W6"]), f64(inp["s6"]), f64(inp["b6"])
    assert (s6 > 0).all()
    W6f = s6[:, None] * W6
    out["w6t"] = W6f.T.reshape(3, 64, 1024).transpose(1, 0, 2) \
        .reshape(64, 3 * 1024).astype(np.float32).copy()
    out["b6"] = b6.reshape(8, 128).T.astype(np.float32).copy()
    W7, s7, b7 = f64(inp["W7"]), f64(inp["s7"]), f64(inp["b7"])
    W7f = s7[:, None] * W7
    W7g, W7x = W7f[:, :1024], W7f[:, 1024:]
    out["w7gt"] = W7g.T.reshape(8, 128, 4, 128).transpose(1, 0, 2, 3) \
        .reshape(128, -1).astype(np.float32).copy()
    out["b7"] = b7.reshape(4, 128).T.astype(np.float32).copy()
    out["w7xt"] = W7x.T.reshape(3, 64, 4, 128).transpose(1, 0, 2, 3) \
        .reshape(64, -1).astype(np.float32).copy()
    W8, s8, b8 = f64(inp["W8"]), f64(inp["s8"]), f64(inp["b8"])
    W8f = s8[:, None] * W8
    out["w8t"] = W8f.T.reshape(4, 128, 2, 128).transpose(1, 0, 2, 3) \
        .reshape(128, -1).astype(np.float32).copy()
    out["b8"] = b8.reshape(2, 128).T.astype(np.float32).copy()
    out["w9t"] = f64(inp["W9"]).T.reshape(2, 128, 63).transpose(1, 0, 2) \
        .reshape(128, 2 * 63).astype(np.float32).copy()
    out["b9"] = f64(inp["b9"])[:, None].astype(np.float32)
    return out


def fold_sample(sample_x, h, split=2):
    """sample_x: (3, N) float32. Returns per-core arrays for half h."""
    x = np.asarray(sample_x, np.float64)
    xx = (x * x).sum(0)
    N = x.shape[1]
    NH = N // split
    xo = x[:, h * NH:(h + 1) * NH]
    return {
        "xr": np.concatenate([x, -0.5 * xx[None, :]], 0).astype(np.float32),
        "xa": np.concatenate([xo, np.ones((1, NH))], 0).astype(np.float32),
    }


def make_in_maps(inputs, split=2):
    """inputs: reference setup_inputs() dict (numpy). split cores/sample."""
    shared = fold_weights(inputs)
    x = np.asarray(inputs["x"])
    B = x.shape[0]
    in_maps = []
    for c in range(B * split):
        b, h = c // split, c % split
        m = dict(shared)
        m.update(fold_sample(x[b], h, split))
        in_maps.append(m)
    return in_maps


def cast_inputs(in_maps, nc):
    dts = {}
    for alloc in nc.m.functions[0].allocations:
        if isinstance(alloc, mybir.MemoryLocationSet) and alloc.kind == "ExternalInput":
            dts[alloc.memorylocations[0].name] = mybir.dt.np(alloc.dtype)
    outs = []
    for m in in_maps:
        outs.append({k: np.ascontiguousarray(np.asarray(v).astype(dts[k]))
                     for k, v in m.items() if k in dts})
    return outs


# =====================================================================
# Harness entry point
# =====================================================================
_CACHE = {}


def _make_runner(nc, n_cores):
    """Compile-once SPMD runner (mirrors bass2jax.run_bass_via_pjrt but
    caches the jitted executable across kernel() calls)."""
    import jax
    from concourse import bass2jax
    from concourse.bass2jax import _bass_exec_p, partition_id_tensor, \
        install_neuronx_cc_hook

    install_neuronx_cc_hook()
    partition_name = nc.partition_id_tensor.name if nc.partition_id_tensor else None
    in_names, out_names, out_avals, zero_shapes = [], [], [], []
    for alloc in nc.m.functions[0].allocations:
        if not isinstance(alloc, mybir.MemoryLocationSet):
            continue
        name = alloc.memorylocations[0].name
        if alloc.kind == "ExternalInput":
            if name != partition_name:
                in_names.append(name)
        elif alloc.kind == "ExternalOutput":
            shape = tuple(alloc.tensor_shape)
            dtype = mybir.dt.np(alloc.dtype)
            out_names.append(name)
            out_avals.append(jax.core.ShapedArray(shape, dtype))
            zero_shapes.append((shape, dtype))
    n_params = len(in_names)
    all_names = in_names + out_names + ([partition_name] if partition_name else [])
    donate = tuple(range(n_params, n_params + len(out_names)))

    def _body(*args):
        operands = list(args)
        if partition_name is not None:
            operands.append(partition_id_tensor())
        return tuple(_bass_exec_p.bind(
            *operands, out_avals=tuple(out_avals), in_names=tuple(all_names),
            out_names=tuple(out_names), lowering_input_output_aliases=(),
            sim_require_finite=True, sim_require_nnan=True, nc=nc))

    from jax.experimental.shard_map import shard_map
    from jax.sharding import Mesh, PartitionSpec
    mesh = Mesh(np.asarray(jax.devices()[:n_cores]), ("core",))
    in_specs = (PartitionSpec("core"),) * (n_params + len(out_names))
    out_specs = (PartitionSpec("core"),) * len(out_names)
    jf = jax.jit(
        shard_map(_body, mesh=mesh, in_specs=in_specs, out_specs=out_specs,
                  check_rep=False),
        donate_argnums=donate, keep_unused=True)

    import hashlib
    dev_cache = {}

    def _zeros_dev():
        return [jax.numpy.zeros((n_cores * shape[0],) + shape[1:], dtype)
                for shape, dtype in zero_shapes]

    def run(in_maps):
        h = hashlib.md5()
        for name in in_names:
            for m in in_maps:
                h.update(np.asarray(m[name]).tobytes())
        key = h.hexdigest()
        if key not in dev_cache:
            dev_cache.clear()
            arrs = [np.concatenate([np.asarray(m[name]) for m in in_maps], axis=0)
                    for name in in_names]
            dev_cache[key] = [jax.device_put(a) for a in arrs]
        args = list(dev_cache[key]) + _zeros_dev()
        outs = jf(*args)
        return [{n: np.asarray(outs[i]).reshape((n_cores,) + zero_shapes[i][0])[c]
                 for i, n in enumerate(out_names)}
                for c in range(n_cores)]

    return run


def kernel(**inputs):
    """DGCNN forward. inputs keyed as reference.setup_inputs(); returns
    (B, 63, N) float32. Two NeuronCores per sample (split halves)."""
    from concourse.bass_utils import run_bass_kernel_spmd

    x = np.asarray(inputs["x"])
    B, _, N = x.shape
    SPLIT = 2
    n_cores = B * SPLIT
    key = (B, N)
    if key not in _CACHE:
        nc = build_core(N=N, split=SPLIT)
        runner = None
        try:
            runner = _make_runner(nc, n_cores)
        except Exception:
            runner = None
        _CACHE[key] = (nc, runner)
    nc, runner = _CACHE[key]
    in_maps = cast_inputs(make_in_maps(inputs, split=SPLIT), nc)
    NH = N // SPLIT

    def assemble(res_list):
        full = np.zeros((B, 63, N), np.float32)
        for c in range(n_cores):
            b, h = c // SPLIT, c % SPLIT
            full[b][:, h * NH:(h + 1) * NH] = res_list[c]["out"]
        return full

    if runner is not None:
        try:
            return assemble(runner(in_maps))
        except Exception:
            _CACHE[key] = (nc, None)
    res = run_bass_kernel_spmd(nc, in_maps, core_ids=list(range(n_cores)))
    return assemble(res.results)
